# revision 6
# baseline (speedup 1.0000x reference)
"""Dense linear layer out = x @ W.T + b on 8 Trainium2 NeuronCores.

Strategy: data-parallel over the batch dim (8192/8 = 1024 rows per core),
W replicated. Host pre-casts both operands to bf16 and lays them out
contraction-major (xt = x_shard.T, wt = W.T) so every DMA is contiguous and
the TensorE contraction dim lands on SBUF partitions. The device kernel is a
tiled matmul: x-shard resident in SBUF (8 MB bf16), W streamed once (32 MB)
as per-n-slab SBUF-cached slabs, fp32 accumulation in PSUM, bias added on
PSUM eviction, fp32 output.

Per-core: M=1024, K=4096, N=4096 -> 2048 matmuls of [128x128]@[128x512].
Each n-slab is processed as two half-groups of 4 output-row blocks so the
PSUM evictions of one half hide under the other half's matmuls.
"""

import numpy as np
import ml_dtypes

B, IN, OUT = 8192, 4096, 4096
NCORES = 8
MS = B // NCORES  # 1024 batch rows per core

P = 128
NF = 512            # matmul moving free dim (one PSUM bank of fp32)
KT = IN // P        # 32 contraction tiles
MT = MS // P        # 8 stationary tiles (output partition blocks)
NS = OUT // NF      # 8 output column slabs
HALF = MT // 2      # m-tiles per half-group

SLAB_CHUNK = 4      # k-tiles per wt slab DMA for prefetched slabs
OUT_BUFS = 8

_cache = {}


def _build():
    import concourse.mybir as mybir
    import concourse.tile as tile
    from concourse import bacc

    nc = bacc.Bacc("TRN2", target_bir_lowering=False, debug=False,
                   num_devices=NCORES)
    xt = nc.dram_tensor("xt", [IN, MS], mybir.dt.bfloat16, kind="ExternalInput")
    wt = nc.dram_tensor("wt", [IN, OUT], mybir.dt.bfloat16, kind="ExternalInput")
    bb = nc.dram_tensor("bb", [P, OUT], mybir.dt.float32, kind="ExternalInput")
    out = nc.dram_tensor("out", [MS, OUT], mybir.dt.float32, kind="ExternalOutput")

    xt_t = xt[:].rearrange("(kt p) m -> p kt m", p=P)    # [128, KT, MS]
    wt_t = wt[:].rearrange("(kt p) n -> p kt n", p=P)    # [128, KT, OUT]
    out_t = out[:].rearrange("(mt p) n -> p mt n", p=P)  # [128, MT, OUT]

    with tile.TileContext(nc) as tc:
        with (
            tc.tile_pool(name="xres", bufs=1) as xres_pool,
            tc.tile_pool(name="bias", bufs=1) as bias_pool,
            tc.tile_pool(name="wts", bufs=2) as wts_pool,
            tc.tile_pool(name="psum", bufs=8, space="PSUM") as psum_pool,
            tc.tile_pool(name="outp", bufs=OUT_BUFS) as out_pool,
        ):
            xres = xres_pool.tile([P, KT, MS], mybir.dt.bfloat16)
            bias = bias_pool.tile([P, OUT], mybir.dt.float32)

            def prefetch_slab(ns):
                nslc = slice(ns * NF, (ns + 1) * NF)
                slab = wts_pool.tile([P, KT, NF], mybir.dt.bfloat16,
                                     name="wslab", tag="wslab")
                if ns == 0:
                    # fine-grained + interleaved with the x-shard load so the
                    # first matmuls wait on one k-tile of each, not the lot
                    for k in range(KT):
                        nc.sync.dma_start(xres[:, k], xt_t[:, k])
                        nc.scalar.dma_start(slab[:, k], wt_t[:, k, nslc])
                else:
                    for kc in range(0, KT, SLAB_CHUNK):
                        nc.scalar.dma_start(
                            slab[:, kc:kc + SLAB_CHUNK],
                            wt_t[:, kc:kc + SLAB_CHUNK, nslc])
                return slab

            slab_cur = prefetch_slab(0)
            # bias is first needed by the ns=0 evictions (~30us in); queue it
            # on the scalar ring behind the ns=0 slab so it never competes
            # with the startup-critical loads
            nc.scalar.dma_start(bias[:], bb[:])

            for ns in range(NS):
                nslc = slice(ns * NF, (ns + 1) * NF)
                slab_next = prefetch_slab(ns + 1) if ns + 1 < NS else None
                for half in range(2):
                    ms = range(half * HALF, (half + 1) * HALF)
                    psums = [psum_pool.tile([P, NF], mybir.dt.float32,
                                            name="ps", tag="ps")
                             for _ in ms]
                    for k in range(KT):
                        for i, m in enumerate(ms):
                            nc.tensor.matmul(
                                psums[i][:],
                                xres[:, k, m * P:(m + 1) * P],
                                slab_cur[:, k],
                                start=(k == 0),
                                stop=(k == KT - 1),
                            )
                    for i, m in enumerate(ms):
                        ot = out_pool.tile([P, NF], mybir.dt.float32,
                                           name="ot", tag="ot")
                        nc.vector.tensor_add(ot[:], psums[i][:], bias[:, nslc])
                        nc.sync.dma_start(out_t[:, m, nslc], ot[:])
                slab_cur = slab_next

    nc.compile()
    return nc


def kernel(x, W, b):
    from concourse.bass_utils import run_bass_kernel_spmd

    nc = _cache.get("nc")
    if nc is None:
        nc = _cache["nc"] = _build()

    bf16 = ml_dtypes.bfloat16
    x = np.asarray(x, dtype=np.float32)
    W = np.asarray(W, dtype=np.float32)
    b = np.asarray(b, dtype=np.float32)

    Wt = np.ascontiguousarray(W.astype(bf16).T)                       # [IN, OUT]
    bias = np.ascontiguousarray(
        np.broadcast_to(b.astype(np.float32)[None, :], (P, OUT)))
    xb = x.astype(bf16)

    in_maps = []
    for c in range(NCORES):
        xs = np.ascontiguousarray(xb[c * MS:(c + 1) * MS].T)          # [IN, MS]
        in_maps.append({"xt": xs, "wt": Wt, "bb": bias})

    res = run_bass_kernel_spmd(nc, in_maps, list(range(NCORES)))
    return np.concatenate(
        [res.results[c]["out"] for c in range(NCORES)], axis=0)


# revision 7
# speedup vs baseline: 1.0331x; 1.0331x over previous
"""Dense linear layer out = x @ W.T + b on 8 Trainium2 NeuronCores.

Strategy: data-parallel over the batch dim (8192/8 = 1024 rows per core),
W replicated. Host pre-casts both operands to bf16 and lays them out
contraction-major (xt = x_shard.T, wt = W.T) so every DMA is contiguous and
the TensorE contraction dim lands on SBUF partitions. The device kernel is a
tiled matmul: x-shard resident in SBUF (8 MB bf16), W streamed once (32 MB)
as per-n-slab SBUF-cached slabs, fp32 accumulation in PSUM, bias added on
PSUM eviction, fp32 output.

Per-core: M=1024, K=4096, N=4096 -> 2048 matmuls of [128x128]@[128x512].
Each n-slab is processed as two half-groups of 4 output-row blocks so the
PSUM evictions of one half hide under the other half's matmuls.
"""

import numpy as np
import ml_dtypes

B, IN, OUT = 8192, 4096, 4096
NCORES = 8
MS = B // NCORES  # 1024 batch rows per core

P = 128
NF = 512            # matmul moving free dim (one PSUM bank of fp32)
KT = IN // P        # 32 contraction tiles
MT = MS // P        # 8 stationary tiles (output partition blocks)
NS = OUT // NF      # 8 output column slabs
HALF = MT // 2      # m-tiles per half-group

SLAB_CHUNK = 4      # k-tiles per wt slab DMA for prefetched slabs
OUT_BUFS = 8

_cache = {}


def _build():
    import concourse.mybir as mybir
    import concourse.tile as tile
    from concourse import bacc

    nc = bacc.Bacc("TRN2", target_bir_lowering=False, debug=False,
                   num_devices=NCORES)
    xt = nc.dram_tensor("xt", [IN, MS], mybir.dt.bfloat16, kind="ExternalInput")
    wt = nc.dram_tensor("wt", [IN, OUT], mybir.dt.bfloat16, kind="ExternalInput")
    bb = nc.dram_tensor("bb", [P, OUT], mybir.dt.float32, kind="ExternalInput")
    out = nc.dram_tensor("out", [MS, OUT], mybir.dt.float32, kind="ExternalOutput")

    xt_t = xt[:].rearrange("(kt p) m -> p kt m", p=P)    # [128, KT, MS]
    wt_t = wt[:].rearrange("(kt p) n -> p kt n", p=P)    # [128, KT, OUT]
    out_t = out[:].rearrange("(mt p) n -> p mt n", p=P)  # [128, MT, OUT]

    with tile.TileContext(nc) as tc:
        with (
            tc.tile_pool(name="xres", bufs=1) as xres_pool,
            tc.tile_pool(name="bias", bufs=1) as bias_pool,
            tc.tile_pool(name="wts", bufs=2) as wts_pool,
            tc.tile_pool(name="psum", bufs=8, space="PSUM") as psum_pool,
            tc.tile_pool(name="outp", bufs=OUT_BUFS) as out_pool,
        ):
            xres = xres_pool.tile([P, KT, MS], mybir.dt.bfloat16)
            bias = bias_pool.tile([P, OUT], mybir.dt.float32)

            def prefetch_slab(ns):
                nslc = slice(ns * NF, (ns + 1) * NF)
                slab = wts_pool.tile([P, KT, NF], mybir.dt.bfloat16,
                                     name="wslab", tag="wslab")
                if ns == 0:
                    # fine-grained + interleaved with the x-shard load so the
                    # first matmuls wait on one k-tile of each, not the lot
                    for k in range(KT):
                        nc.sync.dma_start(xres[:, k], xt_t[:, k])
                        nc.scalar.dma_start(slab[:, k], wt_t[:, k, nslc])
                else:
                    for kc in range(0, KT, SLAB_CHUNK):
                        nc.scalar.dma_start(
                            slab[:, kc:kc + SLAB_CHUNK],
                            wt_t[:, kc:kc + SLAB_CHUNK, nslc])
                return slab

            slab_cur = prefetch_slab(0)
            # bias is first needed by the ns=0 evictions (~30us in); queue it
            # on the scalar ring behind the ns=0 slab so it never competes
            # with the startup-critical loads
            nc.scalar.dma_start(bias[:], bb[:])

            for ns in range(NS):
                nslc = slice(ns * NF, (ns + 1) * NF)
                slab_next = prefetch_slab(ns + 1) if ns + 1 < NS else None
                # ns=0 is DMA-supply-limited (x-shard load streams alongside
                # it): one full 8-bank group halves its per-k DMA demand.
                # Later slabs run from SBUF, so two half-groups let each
                # half's PSUM evictions hide under the other half's matmuls.
                groups = [range(MT)] if ns == 0 else [
                    range(h * HALF, (h + 1) * HALF) for h in range(2)]
                for ms in groups:
                    psums = [psum_pool.tile([P, NF], mybir.dt.float32,
                                            name="ps", tag="ps")
                             for _ in ms]
                    for k in range(KT):
                        for i, m in enumerate(ms):
                            nc.tensor.matmul(
                                psums[i][:],
                                xres[:, k, m * P:(m + 1) * P],
                                slab_cur[:, k],
                                start=(k == 0),
                                stop=(k == KT - 1),
                            )
                    for i, m in enumerate(ms):
                        ot = out_pool.tile([P, NF], mybir.dt.float32,
                                           name="ot", tag="ot")
                        nc.vector.tensor_add(ot[:], psums[i][:], bias[:, nslc])
                        nc.sync.dma_start(out_t[:, m, nslc], ot[:])
                slab_cur = slab_next

    nc.compile()
    return nc


def kernel(x, W, b):
    from concourse.bass_utils import run_bass_kernel_spmd

    nc = _cache.get("nc")
    if nc is None:
        nc = _cache["nc"] = _build()

    bf16 = ml_dtypes.bfloat16
    x = np.asarray(x, dtype=np.float32)
    W = np.asarray(W, dtype=np.float32)
    b = np.asarray(b, dtype=np.float32)

    Wt = np.ascontiguousarray(W.astype(bf16).T)                       # [IN, OUT]
    bias = np.ascontiguousarray(
        np.broadcast_to(b.astype(np.float32)[None, :], (P, OUT)))
    xb = x.astype(bf16)

    in_maps = []
    for c in range(NCORES):
        xs = np.ascontiguousarray(xb[c * MS:(c + 1) * MS].T)          # [IN, MS]
        in_maps.append({"xt": xs, "wt": Wt, "bb": bias})

    res = run_bass_kernel_spmd(nc, in_maps, list(range(NCORES)))
    return np.concatenate(
        [res.results[c]["out"] for c in range(NCORES)], axis=0)
